# revision 8
# baseline (speedup 1.0000x reference)
"""Trainium2 Bass kernel for nn_ReallocationMapEncoder.

The reference network is three NAC layers (y = x @ (tanh(W_hat)*sigmoid(M_hat)).T)
applied to a [nsteps, nsyms, nsyms, 3] grid of normalized (t, a, b) indices,
plus a gb broadcast on the trailing axis. NAC is linear in x, so the whole
network collapses to one effective matrix Weff = W3 @ W2 @ W1 of shape [2, 3]:

    y[t, a, b, c] = gb[c] + (t/2)*Weff[c,0] + (a/2048)*Weff[c,1] + (b/2048)*Weff[c,2]

The output [2, 2048, 2048, 2] (67 MB as f32) is a separable affine ramp; the
kernel is purely output-write-bandwidth bound (memory regime).

Device strategy (8 cores, data-parallel on the `a` axis, 256 rows each):
  * emit bf16 on device (the 2e-2 rel-err budget dwarfs bf16's 2^-9 rounding,
    and bf16 keeps f32's exponent range so near-zero outputs stay accurate);
    host upcasts to f32. Halves HBM write traffic vs f32: 4.2 MB/core.
  * the b-index ramp J is generated by a gpsimd iota (f32, exact for ints
    <= 2047) -- it finishes (~5.9us) before the bias input-DMA completion
    (~6.3us, the real gate: any first DMA has ~4us fixed latency on this
    runtime), so the only external input is the tiny bias table.
  * store units are produced by two engines in parallel and written out on
    the two HWDGE rings (no SWDGE -- Q7 descriptor-gen cost up to 8.3us):
      - DVE tensor_scalar, 12 units of [128, 1024] bf16, DMAs issued by the
        otherwise-idle sync (SP) engine with a single producer-sem wait;
      - ACT activation(Identity, bias=per-partition AP), 2 units of
        [128, 2048] (ACT is ~2x slower/elem), DMAs issued by ACT itself in
        program order (zero waits);
    out[p, b, c] = J[b]*(Weff[c,2]/nsyms) + bias[p, (t,blk,c)]
  * the Tile entry all-engine barrier is stripped post-build: every cross-
    engine dependency is carried by monotonic >= sem waits that the runtime
    zeroes per execution, and the barrier otherwise gates the whole kernel
    on the Tensor engine's ~3.2us sequencer boot that nothing here uses.
  * DMAHW lane-recycle waits on output DMAs are dropped post-build: HWDGE
    descriptors on one ring complete in FIFO order, and no instruction waits
    an intermediate value of a recycled output lane -- only the kernel-tail
    drain waits the final counts. Keeping them forced an extra single-wait
    Drain carrier (~0.5us) per DMA on the issuing engine (walrus fits one
    sem wait per HWDGE DMA).
"""

import numpy as np

NSTEPS = 2
NSYMS = 2048
NCORES = 8
A_PER_CORE = NSYMS // NCORES          # 256
BLKS = A_PER_CORE // 128              # 2 partition blocks per core
F = NSYMS * 2                         # 4096 free elements per a-row (b,c interleaved)
DU = 1024                             # DVE store-unit columns
AU = 2048                             # ACT store-unit columns
STRIP_ENTRY_BARRIER = True
DROP_RECYCLE_WAITS = True

_CACHE = {}

# (t, blk, colstart, ncols, engine) store schedule: ACT takes the tail half
# of the (t, 1) row-blocks; DVE covers the rest in 1024-col units, ordered
# q-major so each unit group only needs the iota piece that is already done.
_UNITS = []
for _q in range(4):
    for _t in range(NSTEPS):
        for _blk in range(BLKS):
            if _blk == BLKS - 1 and _q >= 2:
                if _q == 2:
                    _UNITS.append((_t, _blk, 2 * DU, AU, "act"))
            else:
                _UNITS.append((_t, _blk, _q * DU, DU, "dve"))


def _build_bass(scales):
    import concourse.bass as bass
    import concourse.mybir as mybir
    from concourse.tile import TileContext

    f32 = mybir.dt.float32
    bf16 = mybir.dt.bfloat16
    nc = bass.Bass(trn_type="TRN2")

    bias_in = nc.dram_tensor("bias_in", [128, NSTEPS * BLKS * 2], f32, kind="ExternalInput")
    out = nc.dram_tensor("out", [NSTEPS, BLKS, 128, F], bf16, kind="ExternalOutput")

    with TileContext(nc) as tc:
        with (
            tc.tile_pool(name="const", bufs=1) as const,
            tc.tile_pool(name="outp", bufs=len(_UNITS)) as outp,
        ):
            bias_sb = const.tile([128, NSTEPS * BLKS * 2], f32)
            warm_sb = const.tile([1, 1], f32)
            # warm-up DMA: absorbs the DMA subsystem's cold-start cost so the
            # bias load right behind it completes sooner; nobody reads warm_sb
            dw = nc.sync.dma_start(warm_sb[:], bias_in[0:1, 0:1])
            d1 = nc.sync.dma_start(bias_sb[:], bias_in[:])
            hoist_names = [dw.ins.name, d1.ins.name]

            # J ramp in 4 iota pieces so the first DVE/ACT units only gate on
            # an early piece (each [128,512] piece takes ~0.9us on the Q7)
            jtab_sb = const.tile([128, NSYMS], f32)
            NP = 4
            JP = NSYMS // NP
            for k in range(NP):
                nc.gpsimd.iota(
                    jtab_sb[:, k * JP : (k + 1) * JP], pattern=[[1, JP]],
                    base=k * JP, channel_multiplier=0,
                    allow_small_or_imprecise_dtypes=True,
                )

            # Observer copies: fold the bias-DMA lane sem and the gpsimd
            # (Pool) iota sems into each compute engine's vector clock with
            # single-wait instructions, staged so each unit group's ops need
            # no waits of their own. ACT only ever reads pieces 2 and 3.
            vscr = const.tile([1, NP + 1], f32)
            sscr = const.tile([1, 3], f32)
            nc.vector.tensor_copy(vscr[:, 0:1], bias_sb[0:1, 0:1])
            nc.scalar.copy(sscr[:, 0:1], bias_sb[0:1, 0:1])
            nc.scalar.copy(sscr[:, 1:2], jtab_sb[0:1, 2 * JP : 2 * JP + 1])
            nc.scalar.copy(sscr[:, 2:3], jtab_sb[0:1, 3 * JP : 3 * JP + 1])

            seen_piece = -1
            for t, blk, col0, ncols, eng in _UNITS:
                need_piece = (col0 // 2 + ncols // 2 - 1) // JP
                if eng == "dve" and need_piece > seen_piece:
                    nc.vector.tensor_copy(
                        vscr[:, need_piece + 1 : need_piece + 2],
                        jtab_sb[0:1, need_piece * JP : need_piece * JP + 1],
                    )
                    seen_piece = need_piece
                ot = outp.tile([128, ncols], bf16)
                otv = ot[:].rearrange("p (b c) -> p b c", c=2)
                jsl = jtab_sb[:, col0 // 2 : col0 // 2 + ncols // 2]
                for c in range(2):
                    idx = (t * BLKS + blk) * 2 + c
                    bap = bias_sb[:, idx : idx + 1]
                    if eng == "act":
                        nc.scalar.activation(
                            otv[:, :, c], jsl,
                            mybir.ActivationFunctionType.Identity,
                            bias=bap, scale=scales[c],
                        )
                    else:
                        nc.vector.tensor_scalar(
                            otv[:, :, c], jsl, scales[c], bap,
                            mybir.AluOpType.mult, mybir.AluOpType.add,
                        )
                dst = out[t, blk][:, col0 : col0 + ncols]
                if eng == "act":
                    nc.scalar.dma_start(dst, ot[:])
                else:
                    nc.sync.dma_start(dst, ot[:])

    if STRIP_ENTRY_BARRIER:
        _strip_entry_barrier(nc, mybir)
    _hoist_input_dmas(nc, mybir, hoist_names)
    if DROP_RECYCLE_WAITS:
        _drop_recycle_waits(nc, mybir)
    _legalize_waits(nc, mybir)
    return nc


def _strip_entry_barrier(nc, mybir):
    """Remove the all-engine start barrier (both butterfly phases) and its
    paired Drains from the NEFF entry block. All kernel dependencies are
    monotonic >= waits on runtime-zeroed sems, so engines can start their
    streams immediately; the barrier only serialized everyone behind the
    slowest engine's (Tensor, unused here) ~3us sequencer boot. The exit
    barrier is kept."""
    entry = nc.m.functions[0].blocks[0]
    keep = []
    for inst in entry.instructions:
        if isinstance(inst, mybir.InstEventSemaphore) and inst.name.startswith(
            "barrier_"
        ):
            continue
        if isinstance(inst, mybir.InstDrain):
            continue
        keep.append(inst)
    entry.instructions = keep


def _hoist_input_dmas(nc, mybir, names):
    """Move the (dependency-free) input-load DMAs from the tile block into
    the NEFF entry block so they dispatch as early as possible. Sems only
    fire EARLIER, so all downstream waits stay correct."""
    func = nc.m.functions[0]
    entry = func.blocks[0]
    moved = []
    for block in func.blocks[1:]:
        keep = []
        for inst in block.instructions:
            if inst.name in names:
                moved.append(inst)
            else:
                keep.append(inst)
        if len(keep) != len(block.instructions):
            block.instructions = keep
    assert len(moved) == len(names), (len(moved), names)
    moved.sort(key=lambda i: names.index(i.name))
    insts = list(entry.instructions)
    pos = len(insts)
    for k, inst in enumerate(insts):
        if inst.engine == mybir.EngineType.SP and isinstance(
            inst, (mybir.InstDrain, mybir.InstUnconditionalBranch)
        ):
            pos = k
            break
    entry.instructions = insts[:pos] + moved + insts[pos:]


def _drop_recycle_waits(nc, mybir):
    """Output DMAs whose DMAHW completion lane is recycled get a second
    'previous lane user done' wait from Tile. Descriptors on one HWDGE ring
    complete in FIFO order and nothing waits intermediate values of
    recycled output lanes (the kernel-tail drain waits the final counts),
    so the wait is redundant -- and walrus only fits ONE wait per HWDGE
    DMA, forcing a costly extra Drain carrier. Drop DMAHW-sem waits from
    any DMA that also carries a producer-engine wait, and from ACT-issued
    DMAs ordered by program order."""
    func = nc.m.functions[0]
    for block in func.blocks:
        for inst in block.instructions:
            if not isinstance(inst, mybir.InstDMACopy):
                continue
            si = inst.sync_info
            waits = list(si.on_wait) if si is not None and si.on_wait else []
            if not waits:
                continue
            kept = [w for w in waits if not str(getattr(w, "ant_name", "")).startswith("DMAHW")]
            if len(kept) != len(waits):
                inst.sync_info = mybir.SyncInfo(
                    on_wait=kept, on_update=list(si.on_update or [])
                )


def _legalize_waits(nc, mybir):
    """This walrus build fits very few semaphore waits per instruction (one
    for most engine structs). Tile's auto-generated kernel-tail drain waits
    on every DMA lane + engine sem at once; split any multi-wait instruction
    into a chain of single-wait Drain carriers on the same engine."""
    for func in nc.m.functions:
        for block in func.blocks:
            insts = list(block.instructions)
            new_insts = []
            changed = False
            for inst in insts:
                si = inst.sync_info
                waits = list(si.on_wait) if si is not None and si.on_wait else []
                if len(waits) > 1:
                    for w in waits[:-1]:
                        d = mybir.InstDrain(
                            name=f"{inst.name}-waitsplit-{len(new_insts)}",
                            ins=[],
                            outs=[],
                            bass_is_fusable=False,
                        )
                        d.engine = inst.engine
                        d.sync_info = mybir.SyncInfo(on_wait=[w], on_update=[])
                        new_insts.append(d)
                    inst.sync_info = mybir.SyncInfo(
                        on_wait=[waits[-1]], on_update=list(si.on_update or [])
                    )
                    changed = True
                new_insts.append(inst)
            if changed:
                block.instructions = new_insts


def _host_consts(gb, w_hat1, m_hat1, w_hat2, m_hat2, w_hat3, m_hat3):
    def nacw(w, m):
        w = np.asarray(w, np.float64)
        m = np.asarray(m, np.float64)
        return np.tanh(w) * (1.0 / (1.0 + np.exp(-m)))

    weff = nacw(w_hat3, m_hat3) @ nacw(w_hat2, m_hat2) @ nacw(w_hat1, m_hat1)  # [2,3]
    gb = np.asarray(gb, np.float64)

    scales = [float(np.float32(weff[c, 2] / NSYMS)) for c in range(2)]

    # bias[core][p, (t,blk,c)] = gb[c] + (t/2)Weff[c,0] + (a/2048)Weff[c,1]
    biases = []
    for core in range(NCORES):
        bias = np.empty((128, NSTEPS, BLKS, 2), np.float64)
        for t in range(NSTEPS):
            for blk in range(BLKS):
                a = (core * A_PER_CORE + blk * 128 + np.arange(128)) / NSYMS
                for c in range(2):
                    bias[:, t, blk, c] = (
                        gb[c] + (t / NSTEPS) * weff[c, 0] + a * weff[c, 1]
                    )
        biases.append(np.ascontiguousarray(bias.reshape(128, -1), np.float32))
    return scales, biases


def kernel(market, gb, w_hat1, m_hat1, w_hat2, m_hat2, w_hat3, m_hat3):
    from concourse.bass_utils import run_bass_kernel_spmd

    scales, biases = _host_consts(
        gb, w_hat1, m_hat1, w_hat2, m_hat2, w_hat3, m_hat3
    )
    # the tensor_scalar immediates (scales) are baked into the traced program,
    # so the compiled module is keyed on them
    key = ("nc", tuple(scales))
    if key not in _CACHE:
        _CACHE[key] = _build_bass(scales)
    nc = _CACHE[key]
    _CACHE["last_nc"] = nc

    in_maps = [{"bias_in": biases[core]} for core in range(NCORES)]
    res = run_bass_kernel_spmd(nc, in_maps, core_ids=list(range(NCORES)))
    parts = [
        np.asarray(r["out"]).reshape(NSTEPS, A_PER_CORE, NSYMS, 2).astype(np.float32)
        for r in res.results
    ]
    return np.concatenate(parts, axis=1)


# revision 11
# speedup vs baseline: 1.0015x; 1.0015x over previous
"""Trainium2 Bass kernel for nn_ReallocationMapEncoder.

The reference network is three NAC layers (y = x @ (tanh(W_hat)*sigmoid(M_hat)).T)
applied to a [nsteps, nsyms, nsyms, 3] grid of normalized (t, a, b) indices,
plus a gb broadcast on the trailing axis. NAC is linear in x, so the whole
network collapses to one effective matrix Weff = W3 @ W2 @ W1 of shape [2, 3]:

    y[t, a, b, c] = gb[c] + (t/2)*Weff[c,0] + (a/2048)*Weff[c,1] + (b/2048)*Weff[c,2]

The output [2, 2048, 2048, 2] (67 MB as f32) is a separable affine ramp; the
kernel is purely output-write-bandwidth bound (memory regime).

Device strategy (8 cores, data-parallel on the `a` axis, 256 rows each):
  * emit bf16 on device (the 2e-2 rel-err budget dwarfs bf16's 2^-9 rounding,
    and bf16 keeps f32's exponent range so near-zero outputs stay accurate);
    host upcasts to f32. Halves HBM write traffic vs f32: 4.2 MB/core.
  * the b-index ramp J is generated by a gpsimd iota (f32, exact for ints
    <= 2047) -- it finishes (~5.9us) before the bias input-DMA completion
    (~6.3us, the real gate: any first DMA has ~4us fixed latency on this
    runtime), so the only external input is the tiny bias table.
  * store units are produced by two engines in parallel and written out on
    the two HWDGE rings (no SWDGE -- Q7 descriptor-gen cost up to 8.3us):
      - DVE tensor_scalar, 12 units of [128, 1024] bf16, DMAs issued by the
        otherwise-idle sync (SP) engine with a single producer-sem wait;
      - ACT activation(Identity, bias=per-partition AP), 2 units of
        [128, 2048] (ACT is ~2x slower/elem), DMAs issued by ACT itself in
        program order (zero waits);
    out[p, b, c] = J[b]*(Weff[c,2]/nsyms) + bias[p, (t,blk,c)]
  * the Tile entry all-engine barrier is stripped post-build: every cross-
    engine dependency is carried by monotonic >= sem waits that the runtime
    zeroes per execution, and the barrier otherwise gates the whole kernel
    on the Tensor engine's ~3.2us sequencer boot that nothing here uses.
  * DMAHW lane-recycle waits on output DMAs are dropped post-build: HWDGE
    descriptors on one ring complete in FIFO order, and no instruction waits
    an intermediate value of a recycled output lane -- only the kernel-tail
    drain waits the final counts. Keeping them forced an extra single-wait
    Drain carrier (~0.5us) per DMA on the issuing engine (walrus fits one
    sem wait per HWDGE DMA).
"""

import numpy as np

NSTEPS = 2
NSYMS = 2048
NCORES = 8
A_PER_CORE = NSYMS // NCORES          # 256
BLKS = A_PER_CORE // 128              # 2 partition blocks per core
F = NSYMS * 2                         # 4096 free elements per a-row (b,c interleaved)
DU = 1024                             # DVE store-unit columns
AU = 2048                             # ACT store-unit columns
STRIP_ENTRY_BARRIER = True
STRIP_EXIT_BARRIER = False
STRIP_PE = False
DROP_RECYCLE_WAITS = True

_CACHE = {}

# (t, blk, colstart, ncols, engine) store schedule. The J iota lands in two
# [128,1024] pieces; units that only need piece 0 (cols < 2048) go first on
# each engine. ACT (slower/elem) gets the blk-1 head halves (piece-0 only, so
# it starts as soon as the bias lands); DVE covers the rest in 1024-col
# units: first the piece-0 group, then the piece-1 group.
_UNITS = []
for _t in range(NSTEPS):
    for _q in range(2):
        _UNITS.append((_t, 0, _q * DU, DU, "dve"))
for _t in range(NSTEPS):
    _UNITS.append((_t, 1, 0, AU, "act"))
for _t in range(NSTEPS):
    for _q in range(2, 4):
        _UNITS.append((_t, 0, _q * DU, DU, "dve"))
for _t in range(NSTEPS):
    for _q in range(2, 4):
        _UNITS.append((_t, 1, _q * DU, DU, "dve"))


def _build_bass(scales):
    import concourse.bass as bass
    import concourse.mybir as mybir
    from concourse.tile import TileContext

    f32 = mybir.dt.float32
    bf16 = mybir.dt.bfloat16
    nc = bass.Bass(trn_type="TRN2")

    bias_in = nc.dram_tensor("bias_in", [128, NSTEPS * BLKS * 2], f32, kind="ExternalInput")
    out = nc.dram_tensor("out", [NSTEPS, BLKS, 128, F], bf16, kind="ExternalOutput")

    with TileContext(nc) as tc:
        with (
            tc.tile_pool(name="const", bufs=1) as const,
            tc.tile_pool(name="outp", bufs=len(_UNITS)) as outp,
        ):
            bias_sb = const.tile([128, NSTEPS * BLKS * 2], f32)
            warm_sb = const.tile([1, 1], f32)
            # warm-up DMA: absorbs the DMA subsystem's cold-start cost so the
            # bias load right behind it completes sooner; nobody reads warm_sb
            dw = nc.sync.dma_start(warm_sb[:], bias_in[0:1, 0:1])
            d1 = nc.sync.dma_start(bias_sb[:], bias_in[:])
            hoist_names = [dw.ins.name, d1.ins.name]

            # J ramp in 2 iota pieces so piece-0 units only gate on ~half the
            # Q7 iota time. No observer copies: Tile's scheduler reorders
            # them to the stream head (serializing the engine behind ALL
            # deps); instead the group-leading compute ops carry their <=2
            # sem waits directly, and _legalize_waits splits any multi-wait
            # op into an in-place drain + op that the scheduler cannot move.
            jtab_sb = const.tile([128, NSYMS], f32)
            JP = NSYMS // 2
            for k in range(2):
                nc.gpsimd.iota(
                    jtab_sb[:, k * JP : (k + 1) * JP], pattern=[[1, JP]],
                    base=k * JP, channel_multiplier=0,
                    allow_small_or_imprecise_dtypes=True,
                )

            for t, blk, col0, ncols, eng in _UNITS:
                ot = outp.tile([128, ncols], bf16)
                otv = ot[:].rearrange("p (b c) -> p b c", c=2)
                jsl = jtab_sb[:, col0 // 2 : col0 // 2 + ncols // 2]
                for c in range(2):
                    idx = (t * BLKS + blk) * 2 + c
                    bap = bias_sb[:, idx : idx + 1]
                    if eng == "act":
                        nc.scalar.activation(
                            otv[:, :, c], jsl,
                            mybir.ActivationFunctionType.Identity,
                            bias=bap, scale=scales[c],
                        )
                    else:
                        nc.vector.tensor_scalar(
                            otv[:, :, c], jsl, scales[c], bap,
                            mybir.AluOpType.mult, mybir.AluOpType.add,
                        )
                dst = out[t, blk][:, col0 : col0 + ncols]
                if eng == "act":
                    nc.scalar.dma_start(dst, ot[:])
                else:
                    nc.sync.dma_start(dst, ot[:])

    if STRIP_ENTRY_BARRIER:
        _strip_entry_barrier(nc, mybir)
    _hoist_input_dmas(nc, mybir, hoist_names)
    if DROP_RECYCLE_WAITS:
        _drop_recycle_waits(nc, mybir)
    _legalize_waits(nc, mybir)
    return nc


def _strip_entry_barrier(nc, mybir):
    """Remove the all-engine start barrier (both butterfly phases) and its
    paired Drains from the NEFF entry block. All kernel dependencies are
    monotonic >= waits on runtime-zeroed sems, so engines can start their
    streams immediately; the barrier only serialized everyone behind the
    slowest engine's (Tensor, unused here) ~3us sequencer boot. The exit
    barrier is kept."""
    entry = nc.m.functions[0].blocks[0]
    keep = []
    for inst in entry.instructions:
        if isinstance(inst, mybir.InstEventSemaphore) and inst.name.startswith(
            "barrier_"
        ):
            continue
        if isinstance(inst, mybir.InstDrain):
            continue
        keep.append(inst)
    entry.instructions = keep


def _hoist_input_dmas(nc, mybir, names):
    """Move the (dependency-free) input-load DMAs from the tile block into
    the NEFF entry block so they dispatch as early as possible. Sems only
    fire EARLIER, so all downstream waits stay correct."""
    func = nc.m.functions[0]
    entry = func.blocks[0]
    moved = []
    for block in func.blocks[1:]:
        keep = []
        for inst in block.instructions:
            if inst.name in names:
                moved.append(inst)
            else:
                keep.append(inst)
        if len(keep) != len(block.instructions):
            block.instructions = keep
    assert len(moved) == len(names), (len(moved), names)
    moved.sort(key=lambda i: names.index(i.name))
    insts = list(entry.instructions)
    pos = len(insts)
    for k, inst in enumerate(insts):
        if inst.engine == mybir.EngineType.SP and isinstance(
            inst, (mybir.InstDrain, mybir.InstUnconditionalBranch)
        ):
            pos = k
            break
    entry.instructions = insts[:pos] + moved + insts[pos:]


def _drop_recycle_waits(nc, mybir):
    """Output DMAs whose DMAHW completion lane is recycled get a second
    'previous lane user done' wait from Tile. Descriptors on one HWDGE ring
    complete in FIFO order and nothing waits intermediate values of
    recycled output lanes (the kernel-tail drain waits the final counts),
    so the wait is redundant -- and walrus only fits ONE wait per HWDGE
    DMA, forcing a costly extra Drain carrier. Drop DMAHW-sem waits from
    any DMA that also carries a producer-engine wait, and from ACT-issued
    DMAs ordered by program order."""
    func = nc.m.functions[0]
    for block in func.blocks:
        for inst in block.instructions:
            if not isinstance(inst, mybir.InstDMACopy):
                continue
            si = inst.sync_info
            waits = list(si.on_wait) if si is not None and si.on_wait else []
            if not waits:
                continue
            kept = [w for w in waits if not str(getattr(w, "ant_name", "")).startswith("DMAHW")]
            if len(kept) != len(waits):
                inst.sync_info = mybir.SyncInfo(
                    on_wait=kept, on_update=list(si.on_update or [])
                )


def _legalize_waits(nc, mybir):
    """This walrus build fits very few semaphore waits per instruction (one
    for most engine structs). Tile's auto-generated kernel-tail drain waits
    on every DMA lane + engine sem at once; split any multi-wait instruction
    into a chain of single-wait Drain carriers on the same engine."""
    for func in nc.m.functions:
        for block in func.blocks:
            insts = list(block.instructions)
            new_insts = []
            changed = False
            for inst in insts:
                si = inst.sync_info
                waits = list(si.on_wait) if si is not None and si.on_wait else []
                if len(waits) > 1:
                    for w in waits[:-1]:
                        d = mybir.InstDrain(
                            name=f"{inst.name}-waitsplit-{len(new_insts)}",
                            ins=[],
                            outs=[],
                            bass_is_fusable=False,
                        )
                        d.engine = inst.engine
                        d.sync_info = mybir.SyncInfo(on_wait=[w], on_update=[])
                        new_insts.append(d)
                    inst.sync_info = mybir.SyncInfo(
                        on_wait=[waits[-1]], on_update=list(si.on_update or [])
                    )
                    changed = True
                new_insts.append(inst)
            if changed:
                block.instructions = new_insts


def _host_consts(gb, w_hat1, m_hat1, w_hat2, m_hat2, w_hat3, m_hat3):
    def nacw(w, m):
        w = np.asarray(w, np.float64)
        m = np.asarray(m, np.float64)
        return np.tanh(w) * (1.0 / (1.0 + np.exp(-m)))

    weff = nacw(w_hat3, m_hat3) @ nacw(w_hat2, m_hat2) @ nacw(w_hat1, m_hat1)  # [2,3]
    gb = np.asarray(gb, np.float64)

    scales = [float(np.float32(weff[c, 2] / NSYMS)) for c in range(2)]

    # bias[core][p, (t,blk,c)] = gb[c] + (t/2)Weff[c,0] + (a/2048)Weff[c,1]
    biases = []
    for core in range(NCORES):
        bias = np.empty((128, NSTEPS, BLKS, 2), np.float64)
        for t in range(NSTEPS):
            for blk in range(BLKS):
                a = (core * A_PER_CORE + blk * 128 + np.arange(128)) / NSYMS
                for c in range(2):
                    bias[:, t, blk, c] = (
                        gb[c] + (t / NSTEPS) * weff[c, 0] + a * weff[c, 1]
                    )
        biases.append(np.ascontiguousarray(bias.reshape(128, -1), np.float32))
    return scales, biases


def kernel(market, gb, w_hat1, m_hat1, w_hat2, m_hat2, w_hat3, m_hat3):
    from concourse.bass_utils import run_bass_kernel_spmd

    scales, biases = _host_consts(
        gb, w_hat1, m_hat1, w_hat2, m_hat2, w_hat3, m_hat3
    )
    # the tensor_scalar immediates (scales) are baked into the traced program,
    # so the compiled module is keyed on them
    key = ("nc", tuple(scales))
    if key not in _CACHE:
        _CACHE[key] = _build_bass(scales)
    nc = _CACHE[key]
    _CACHE["last_nc"] = nc

    in_maps = [{"bias_in": biases[core]} for core in range(NCORES)]
    res = run_bass_kernel_spmd(nc, in_maps, core_ids=list(range(NCORES)))
    parts = [
        np.asarray(r["out"]).reshape(NSTEPS, A_PER_CORE, NSYMS, 2).astype(np.float32)
        for r in res.results
    ]
    return np.concatenate(parts, axis=1)


# revision 17
# speedup vs baseline: 1.1077x; 1.1061x over previous
"""Trainium2 Bass kernel for nn_ReallocationMapEncoder.

The reference network is three NAC layers (y = x @ (tanh(W_hat)*sigmoid(M_hat)).T)
applied to a [nsteps, nsyms, nsyms, 3] grid of normalized (t, a, b) indices,
plus a gb broadcast on the trailing axis. NAC is linear in x, so the whole
network collapses to one effective matrix Weff = W3 @ W2 @ W1 of shape [2, 3]:

    y[t, a, b, c] = gb[c] + (t/2)*Weff[c,0] + (a/2048)*Weff[c,1] + (b/2048)*Weff[c,2]

The output [2, 2048, 2048, 2] (67 MB as f32) is a separable affine ramp; the
kernel is purely output-write-bandwidth bound (memory regime).

Device strategy (8 cores, data-parallel on the `a` axis, 256 rows each):
  * emit bf16 on device (the 2e-2 rel-err budget dwarfs bf16's 2^-9 rounding,
    and bf16 keeps f32's exponent range so near-zero outputs stay accurate);
    host upcasts to f32. Halves HBM write traffic vs f32: 4.2 MB/core.
  * the b-index ramp J is generated by a gpsimd iota (f32, exact for ints
    <= 2047) -- it finishes (~5.9us) before the bias input-DMA completion
    (~6.3us, the real gate: any first DMA has ~4us fixed latency on this
    runtime), so the only external input is the tiny bias table.
  * store units are produced by two engines in parallel and written out on
    the two HWDGE rings (no SWDGE -- Q7 descriptor-gen cost up to 8.3us):
      - DVE tensor_scalar, 12 units of [128, 1024] bf16, DMAs issued by the
        otherwise-idle sync (SP) engine with a single producer-sem wait;
      - ACT activation(Identity, bias=per-partition AP), 2 units of
        [128, 2048] (ACT is ~2x slower/elem), DMAs issued by ACT itself in
        program order (zero waits);
    out[p, b, c] = J[b]*(Weff[c,2]/nsyms) + bias[p, (t,blk,c)]
  * the Tile entry all-engine barrier is stripped post-build: every cross-
    engine dependency is carried by monotonic >= sem waits that the runtime
    zeroes per execution, and the barrier otherwise gates the whole kernel
    on the Tensor engine's ~3.2us sequencer boot that nothing here uses.
  * DMAHW lane-recycle waits on output DMAs are dropped post-build: HWDGE
    descriptors on one ring complete in FIFO order, and no instruction waits
    an intermediate value of a recycled output lane -- only the kernel-tail
    drain waits the final counts. Keeping them forced an extra single-wait
    Drain carrier (~0.5us) per DMA on the issuing engine (walrus fits one
    sem wait per HWDGE DMA).
"""

import numpy as np

NSTEPS = 2
NSYMS = 2048
NCORES = 8
A_PER_CORE = NSYMS // NCORES          # 256
BLKS = A_PER_CORE // 128              # 2 partition blocks per core
F = NSYMS * 2                         # 4096 free elements per a-row (b,c interleaved)
DU = 1024                             # DVE store-unit columns
AU = 2048                             # ACT store-unit columns
STRIP_ENTRY_BARRIER = True
STRIP_EXIT_BARRIER = False   # breaks NEFF execution (runtime INTERNAL error)
STRIP_PE = False
DROP_RECYCLE_WAITS = True

_CACHE = {}

# (t, blk, colstart, ncols, engine) store schedule: ACT takes the tail half
# of the (t, 1) row-blocks; DVE covers the rest in 1024-col units.
_UNITS = []
for _t in range(NSTEPS):
    for _blk in range(BLKS):
        if _blk == BLKS - 1:
            for _q in range(2):
                _UNITS.append((_t, _blk, _q * DU, DU, "dve"))
            _UNITS.append((_t, _blk, 2 * DU, AU, "act"))
        else:
            for _q in range(4):
                _UNITS.append((_t, _blk, _q * DU, DU, "dve"))


def _build_bass(scales):
    import concourse.bass as bass
    import concourse.mybir as mybir
    from concourse.tile import TileContext

    f32 = mybir.dt.float32
    bf16 = mybir.dt.bfloat16
    nc = bass.Bass(trn_type="TRN2")

    bias_in = nc.dram_tensor("bias_in", [128, NSTEPS * BLKS * 2], f32, kind="ExternalInput")
    out = nc.dram_tensor("out", [NSTEPS, BLKS, 128, F], bf16, kind="ExternalOutput")

    with TileContext(nc) as tc:
        with (
            tc.tile_pool(name="const", bufs=1) as const,
            tc.tile_pool(name="outp", bufs=len(_UNITS)) as outp,
        ):
            bias_sb = const.tile([128, NSTEPS * BLKS * 2], f32)
            warm_sb = const.tile([1, 1], f32)
            # warm-up DMA: absorbs the DMA subsystem's cold-start cost so the
            # bias load right behind it completes sooner; nobody reads warm_sb
            dw = nc.sync.dma_start(warm_sb[:], bias_in[0:1, 0:1])
            d1 = nc.sync.dma_start(bias_sb[:], bias_in[:])
            hoist_names = [dw.ins.name, d1.ins.name]

            # J ramp: the Q7 iota covers only the LOW half (its ~1.8us lands
            # before the bias DMA anyway); ACT derives the high half with one
            # dense Copy(+1024) so the full table is ready ~1.7us earlier
            # than a full-width iota would be.
            jtab_sb = const.tile([128, NSYMS], f32)
            JP = NSYMS // 2
            nc.gpsimd.iota(
                jtab_sb[:, 0:JP], pattern=[[1, JP]], base=0,
                channel_multiplier=0,
                allow_small_or_imprecise_dtypes=True,
            )
            nc.scalar.activation(
                jtab_sb[:, JP : 2 * JP], jtab_sb[:, 0:JP],
                mybir.ActivationFunctionType.Copy, bias=float(JP), scale=1.0,
            )

            # Observer copies: fold the bias-DMA lane sem and the gpsimd
            # (Pool) iota sem into each compute engine's vector clock with
            # single-wait instructions, so most compute ops below and ACT's
            # own DMAs need no waits. (DVE's first high-half unit carries a
            # single ACT-sem wait for the derived table half.)
            vscr = const.tile([1, 2], f32)
            sscr = const.tile([1, 2], f32)
            nc.vector.tensor_copy(vscr[:, 0:1], bias_sb[0:1, 0:1])
            nc.vector.tensor_copy(vscr[:, 1:2], jtab_sb[0:1, 0:1])
            nc.scalar.copy(sscr[:, 0:1], bias_sb[0:1, 0:1])
            nc.scalar.copy(sscr[:, 1:2], jtab_sb[0:1, 0:1])

            for t, blk, col0, ncols, eng in _UNITS:
                ot = outp.tile([128, ncols], bf16)
                otv = ot[:].rearrange("p (b c) -> p b c", c=2)
                jsl = jtab_sb[:, col0 // 2 : col0 // 2 + ncols // 2]
                for c in range(2):
                    idx = (t * BLKS + blk) * 2 + c
                    bap = bias_sb[:, idx : idx + 1]
                    if eng == "act":
                        nc.scalar.activation(
                            otv[:, :, c], jsl,
                            mybir.ActivationFunctionType.Identity,
                            bias=bap, scale=scales[c],
                        )
                    else:
                        nc.vector.tensor_scalar(
                            otv[:, :, c], jsl, scales[c], bap,
                            mybir.AluOpType.mult, mybir.AluOpType.add,
                        )
                dst = out[t, blk][:, col0 : col0 + ncols]
                if eng == "act":
                    nc.scalar.dma_start(dst, ot[:])
                else:
                    nc.sync.dma_start(dst, ot[:])

    if STRIP_ENTRY_BARRIER:
        _strip_entry_barrier(nc, mybir)
    if STRIP_EXIT_BARRIER:
        _strip_exit_barrier(nc, mybir)
    if STRIP_PE:
        _strip_pe(nc, mybir)
    _hoist_input_dmas(nc, mybir, hoist_names)
    if DROP_RECYCLE_WAITS:
        _drop_recycle_waits(nc, mybir)
    _legalize_waits(nc, mybir)
    return nc


def _strip_entry_barrier(nc, mybir):
    """Remove the all-engine start barrier (both butterfly phases) and its
    paired Drains from the NEFF entry block. All kernel dependencies are
    monotonic >= waits on runtime-zeroed sems, so engines can start their
    streams immediately; the barrier only serialized everyone behind the
    slowest engine's (Tensor, unused here) ~3us sequencer boot. The exit
    barrier is kept."""
    entry = nc.m.functions[0].blocks[0]
    keep = []
    for inst in entry.instructions:
        if isinstance(inst, mybir.InstEventSemaphore) and inst.name.startswith(
            "barrier_"
        ):
            continue
        if isinstance(inst, mybir.InstDrain):
            continue
        keep.append(inst)
    entry.instructions = keep


def _strip_exit_barrier(nc, mybir):
    """Remove the all-engine EXIT barrier EventSemaphores (but keep every
    Drain: the kernel-tail drains carry the DMA-lane-final waits that
    guarantee output data has landed). Each engine then halts right after
    its own drain chain instead of rendezvousing (~0.8us) first."""
    for func in nc.m.functions:
        for block in func.blocks[1:]:
            block.instructions = [
                i
                for i in block.instructions
                if not (
                    isinstance(i, mybir.InstEventSemaphore)
                    and (i.name.startswith("barrier_") or i.name.startswith("aeb"))
                )
            ]


def _strip_pe(nc, mybir):
    """Drop every PE (Tensor-engine) instruction: the kernel never uses the
    systolic array, and the runtime's NEFF start barrier otherwise waits
    ~3.4us for the Tensor sequencer to boot."""
    for func in nc.m.functions:
        for block in func.blocks:
            block.instructions = [
                i for i in block.instructions if i.engine != mybir.EngineType.PE
            ]


def _hoist_input_dmas(nc, mybir, names):
    """Move the (dependency-free) input-load DMAs from the tile block into
    the NEFF entry block so they dispatch as early as possible. Sems only
    fire EARLIER, so all downstream waits stay correct."""
    func = nc.m.functions[0]
    entry = func.blocks[0]
    moved = []
    for block in func.blocks[1:]:
        keep = []
        for inst in block.instructions:
            if inst.name in names:
                moved.append(inst)
            else:
                keep.append(inst)
        if len(keep) != len(block.instructions):
            block.instructions = keep
    assert len(moved) == len(names), (len(moved), names)
    moved.sort(key=lambda i: names.index(i.name))
    insts = list(entry.instructions)
    pos = len(insts)
    for k, inst in enumerate(insts):
        if inst.engine == mybir.EngineType.SP and isinstance(
            inst, (mybir.InstDrain, mybir.InstUnconditionalBranch)
        ):
            pos = k
            break
    entry.instructions = insts[:pos] + moved + insts[pos:]


def _drop_recycle_waits(nc, mybir):
    """Output DMAs whose DMAHW completion lane is recycled get a second
    'previous lane user done' wait from Tile. Descriptors on one HWDGE ring
    complete in FIFO order and nothing waits intermediate values of
    recycled output lanes (the kernel-tail drain waits the final counts),
    so the wait is redundant -- and walrus only fits ONE wait per HWDGE
    DMA, forcing a costly extra Drain carrier. Drop DMAHW-sem waits from
    any DMA that also carries a producer-engine wait, and from ACT-issued
    DMAs ordered by program order."""
    func = nc.m.functions[0]
    for block in func.blocks:
        for inst in block.instructions:
            if not isinstance(inst, mybir.InstDMACopy):
                continue
            si = inst.sync_info
            waits = list(si.on_wait) if si is not None and si.on_wait else []
            if not waits:
                continue
            kept = [w for w in waits if not str(getattr(w, "ant_name", "")).startswith("DMAHW")]
            if len(kept) != len(waits):
                inst.sync_info = mybir.SyncInfo(
                    on_wait=kept, on_update=list(si.on_update or [])
                )


def _legalize_waits(nc, mybir):
    """This walrus build fits very few semaphore waits per instruction (one
    for most engine structs). Tile's auto-generated kernel-tail drain waits
    on every DMA lane + engine sem at once; split any multi-wait instruction
    into a chain of single-wait Drain carriers on the same engine."""
    for func in nc.m.functions:
        for block in func.blocks:
            insts = list(block.instructions)
            new_insts = []
            changed = False
            for inst in insts:
                si = inst.sync_info
                waits = list(si.on_wait) if si is not None and si.on_wait else []
                if len(waits) > 1:
                    for w in waits[:-1]:
                        d = mybir.InstDrain(
                            name=f"{inst.name}-waitsplit-{len(new_insts)}",
                            ins=[],
                            outs=[],
                            bass_is_fusable=False,
                        )
                        d.engine = inst.engine
                        d.sync_info = mybir.SyncInfo(on_wait=[w], on_update=[])
                        new_insts.append(d)
                    inst.sync_info = mybir.SyncInfo(
                        on_wait=[waits[-1]], on_update=list(si.on_update or [])
                    )
                    changed = True
                new_insts.append(inst)
            if changed:
                block.instructions = new_insts


def _host_consts(gb, w_hat1, m_hat1, w_hat2, m_hat2, w_hat3, m_hat3):
    def nacw(w, m):
        w = np.asarray(w, np.float64)
        m = np.asarray(m, np.float64)
        return np.tanh(w) * (1.0 / (1.0 + np.exp(-m)))

    weff = nacw(w_hat3, m_hat3) @ nacw(w_hat2, m_hat2) @ nacw(w_hat1, m_hat1)  # [2,3]
    gb = np.asarray(gb, np.float64)

    scales = [float(np.float32(weff[c, 2] / NSYMS)) for c in range(2)]

    # bias[core][p, (t,blk,c)] = gb[c] + (t/2)Weff[c,0] + (a/2048)Weff[c,1]
    biases = []
    for core in range(NCORES):
        bias = np.empty((128, NSTEPS, BLKS, 2), np.float64)
        for t in range(NSTEPS):
            for blk in range(BLKS):
                a = (core * A_PER_CORE + blk * 128 + np.arange(128)) / NSYMS
                for c in range(2):
                    bias[:, t, blk, c] = (
                        gb[c] + (t / NSTEPS) * weff[c, 0] + a * weff[c, 1]
                    )
        biases.append(np.ascontiguousarray(bias.reshape(128, -1), np.float32))
    return scales, biases


def kernel(market, gb, w_hat1, m_hat1, w_hat2, m_hat2, w_hat3, m_hat3):
    from concourse.bass_utils import run_bass_kernel_spmd

    scales, biases = _host_consts(
        gb, w_hat1, m_hat1, w_hat2, m_hat2, w_hat3, m_hat3
    )
    # the tensor_scalar immediates (scales) are baked into the traced program,
    # so the compiled module is keyed on them
    key = ("nc", tuple(scales))
    if key not in _CACHE:
        _CACHE[key] = _build_bass(scales)
    nc = _CACHE[key]
    _CACHE["last_nc"] = nc

    in_maps = [{"bias_in": biases[core]} for core in range(NCORES)]
    res = run_bass_kernel_spmd(nc, in_maps, core_ids=list(range(NCORES)))
    parts = [
        np.asarray(r["out"]).reshape(NSTEPS, A_PER_CORE, NSYMS, 2).astype(np.float32)
        for r in res.results
    ]
    return np.concatenate(parts, axis=1)
